# revision 2
# baseline (speedup 1.0000x reference)
"""Bidirectional Mamba block (B=4, L=1024, D=1024, DI=2048, DS=16) on 8
Trainium2 NeuronCores.

Sharding: one (batch, direction) pair per core - 8 fully data-parallel
shards, no collectives. Host flips the backward direction, sums the
residual and applies the final LayerNorm while gathering.

Per-core schedule (DVE-saturated; scans at the hardware 2cyc/elem floor):
- stage 1: in_proj xh-half in fp8e4 DoubleRow (2x PE) + causal conv on
  DVE (fills the otherwise-idle pre-scan window) + silu + x_proj
- stage 3: z-half (fp8 DR) + silu gate batched in pairs of d-tiles (2
  ACT-table loads per pair), dt head, softplus via exp/ln (table 6),
  16 decay exps per d-tile on ACT, b/m elementwise muls batched 4
  states per wide 2x DVE op (broadcast-stride-0 du), selective scan on
  DVE, state-sum via identity-matmul PSUM accumulation on PE, gate
- out_proj in fp8e4 DoubleRow: first k-half woven under stage 3
  (partials spilled to DRAM), second half + merge as a short tail
- gpsimd stays idle during stage 3: it shares an SBUF port pair with
  the DVE and measurably slows the scans when active
"""

import os
import sys
import types

sys.path.insert(0, "/opt/trn_rl_repo")

import numpy as np
import ml_dtypes

BF16 = ml_dtypes.bfloat16
FP8 = ml_dtypes.float8_e4m3

import concourse.bass as bass
import concourse.mybir as mybir
from concourse.tile import TileContext
from concourse.bass_utils import run_bass_kernel_spmd
from concourse.masks import make_identity

P = 128
B, L, D = 4, 1024, 1024
DI, DS, DC, DR = 2048, 16, 4, 64
ND = DI // P          # 16 d-tiles
NK_D = D // P         # 8 k-tiles over D
NN = D // P           # 8 n-tiles of out_proj output
CH = 512              # psum chunk (free dim)
NCH = L // CH
NCOLS = 7 + DS        # conv_w(4), conv_b, dt_b, D, A(16)
G = 4                 # states per mul-group
NG = DS // G          # 4 groups

# engine split: which s-groups' muls run on gpsimd (rest on DVE, batched)
B_GP_GROUPS = ()          # gpsimd mul offload disabled (hurts DVE)
M_GP_GROUPS = ()

IN_FP8 = True         # in_proj (both halves) in fp8e4 DoubleRow
W_SCALE = 64.0        # fp8 weight scale
Y_SCALE = 8.0         # pre-gate y scale (fp8 opre dynamic range)

F32 = mybir.dt.float32
BF = mybir.dt.bfloat16
F8 = mybir.dt.float8e4
AF = mybir.ActivationFunctionType
OP = mybir.AluOpType

LAST_EXEC_NS = None
LAST_RESULTS = None


def _install_ntff_hook():
    import antenv

    if "antenv.axon_hooks" in sys.modules:
        return
    mod = types.ModuleType("antenv.axon_hooks")
    mod._hook = None
    mod.set_axon_ntff_profile_hook = lambda h: setattr(mod, "_hook", h)
    mod.get_axon_ntff_profile_hook = lambda: mod._hook
    sys.modules["antenv.axon_hooks"] = mod
    antenv.axon_hooks = mod
    try:
        from trn_agent_boot.trn_boot import _ntff_profile_via_ctypes

        mod.set_axon_ntff_profile_hook(
            _ntff_profile_via_ctypes("/opt/axon/libaxon_pjrt.so")
        )
    except Exception:
        pass


def split_excess_waits(nc, max_waits=1):
    """Walrus encodes at most `max_waits` sync-wait commands per instruction."""
    n_extra = 0
    for f in nc.m.functions:
        for bb in f.blocks:
            insts = bb.instructions
            i = 0
            while i < len(insts):
                inst = insts[i]
                si = inst.sync_info
                if si is not None and len(si.on_wait) > max_waits:
                    waits = list(si.on_wait)
                    for j, w in enumerate(waits[max_waits:]):
                        nop = mybir.InstNoOp(
                            name=f"{inst.name}-xw{j}",
                            engine=inst.engine,
                            bass_nofuse=True,
                            sync_info=mybir.SyncInfo(on_wait=[w], on_update=[]),
                        )
                        insts.insert(i, nop)
                        i += 1
                        n_extra += 1
                    inst.sync_info = mybir.SyncInfo(
                        on_wait=waits[:max_waits], on_update=list(si.on_update)
                    )
                i += 1
    return n_extra


def _build_program():
    nc = bass.Bass("TRN2")

    xdt = F8 if IN_FP8 else BF
    xt = nc.dram_tensor("xt", [P, NK_D * L], xdt, kind="ExternalInput")
    w_in = nc.dram_tensor("w_in", [2 * ND, P, NK_D // 2, 2, P], xdt, kind="ExternalInput")
    w_x = nc.dram_tensor("w_x", [P, ND * (DR + 2 * DS)], BF, kind="ExternalInput")
    w_dt = nc.dram_tensor("w_dt", [DR, ND * P], BF, kind="ExternalInput")
    w_out = nc.dram_tensor("w_out", [NN, P, ND // 2, 2, P], F8, kind="ExternalInput")
    chan = nc.dram_tensor("chan", [P, ND * NCOLS], F32, kind="ExternalInput")
    out = nc.dram_tensor("out", [D, L], F32, kind="ExternalOutput")

    xcb_scr = nc.dram_tensor("xcb_scr", [P, ND * L], BF)
    oh1_scr = nc.dram_tensor("oh1_scr", [P, NN, L], BF)
    bc_scr = nc.dram_tensor("bc_scr", [2 * DS, L], BF)

    with TileContext(nc) as tc:
        with tc.tile_pool(name="res", bufs=1) as res:
            Bc = res.tile([P, DS * L], BF, tag="Bc")
            Cc = res.tile([P, DS * L], BF, tag="Cc")
            ident = res.tile([P, P], BF, tag="ident")
            dbc_bf = res.tile([DR + 2 * DS, L], BF, tag="dbcbf")
            chan_all = res.tile([P, ND * NCOLS], F32, tag="chan")
            wx_all = res.tile([P, ND * (DR + 2 * DS)], BF, tag="wx")
            wdt_all = res.tile([DR, ND * P], BF, tag="wdt")
            kx = res.tile([P, NK_D, L], xdt, tag="kx")
            opre = res.tile([P, ND, L], F8, tag="opre")

            make_identity(nc, ident[:])
            nc.sync.dma_start(chan_all[:], chan[:])
            nc.sync.dma_start(wx_all[:], w_x[:])
            nc.sync.dma_start(wdt_all[:], w_dt[:])
            for k in range(NK_D):
                nc.sync.dma_start(
                    kx[:, k, :], xt[:, k * L : (k + 1) * L]
                )

            def cc(m, col):
                return chan_all[:, m * NCOLS + col : m * NCOLS + col + 1]

            with tc.tile_pool(name="wi", bufs=3) as wip:
                # ---- stage 1: in_proj xh-half + conv(DVE) + silu + x_proj --
                with tc.tile_pool(name="s1", bufs=4) as s1p, \
                     tc.tile_pool(name="s1b", bufs=2) as s1q, \
                     tc.tile_pool(name="psh", bufs=3, space="PSUM") as pshp, \
                     tc.tile_pool(name="ps2", bufs=2, space="PSUM") as ps2p:
                    psx = [
                        ps2p.tile([DR + 2 * DS, CH], F32, tag="psx", name=f"psx{c}")
                        for c in range(NCH)
                    ]
                    for m in range(ND):
                        xh = s1q.tile([P, 3 + L], BF, tag="xh")
                        nc.gpsimd.memset(xh[:, 0:3], 0.0)
                        wi = wip.tile([P, NK_D // 2, 2, P], xdt, tag="wi", name=f"wia{m}")
                        nc.sync.dma_start(wi[:], w_in[m])
                        for c in range(NCH):
                            ps = pshp.tile([P, CH], F32, tag="psh")
                            if IN_FP8:
                                for kp in range(NK_D // 2):
                                    nc.tensor.matmul(
                                        ps[:],
                                        lhsT=wi[:, kp],
                                        rhs=kx[:, 2 * kp : 2 * kp + 2, c * CH : (c + 1) * CH],
                                        start=(kp == 0),
                                        stop=(kp == NK_D // 2 - 1),
                                        perf_mode=mybir.MatmulPerfMode.DoubleRow,
                                    )
                            else:
                                for k in range(NK_D):
                                    nc.tensor.matmul(
                                        ps[:],
                                        lhsT=wi[:, k // 2, k % 2],
                                        rhs=kx[:, k, c * CH : (c + 1) * CH],
                                        start=(k == 0),
                                        stop=(k == NK_D - 1),
                                    )
                            nc.scalar.activation(
                                xh[:, 3 + c * CH : 3 + (c + 1) * CH], ps[:], AF.Copy,
                                scale=(1.0 / W_SCALE) if IN_FP8 else 1.0,
                            )
                        # causal conv on DVE: acc = xh0*w0 + conv_b, 3 fused FMAs
                        acc0 = s1p.tile([P, L], F32, tag="accmid", name=f"ac0_{m}")
                        nc.vector.tensor_scalar(
                            out=acc0[:], in0=xh[:, 0:L],
                            scalar1=cc(m, 0), scalar2=cc(m, 4),
                            op0=OP.mult, op1=OP.add,
                        )
                        acc1 = s1p.tile([P, L], F32, tag="accmid", name=f"ac1_{m}")
                        nc.vector.scalar_tensor_tensor(
                            out=acc1[:], in0=xh[:, 1 : 1 + L], scalar=cc(m, 1),
                            in1=acc0[:], op0=OP.mult, op1=OP.add,
                        )
                        acc2 = s1p.tile([P, L], F32, tag="accmid", name=f"ac2_{m}")
                        nc.vector.scalar_tensor_tensor(
                            out=acc2[:], in0=xh[:, 2 : 2 + L], scalar=cc(m, 2),
                            in1=acc1[:], op0=OP.mult, op1=OP.add,
                        )
                        acc3 = s1p.tile([P, L], F32, tag="acc3", name=f"ac3_{m}")
                        nc.vector.scalar_tensor_tensor(
                            out=acc3[:], in0=xh[:, 3 : 3 + L], scalar=cc(m, 3),
                            in1=acc2[:], op0=OP.mult, op1=OP.add,
                        )
                        xcb = s1q.tile([P, L], BF, tag="xcb")
                        for c in range(NCH):
                            nc.scalar.activation(
                                xcb[:, c * CH : (c + 1) * CH],
                                acc3[:, c * CH : (c + 1) * CH], AF.Silu,
                            )
                            nc.tensor.matmul(
                                psx[c][:],
                                lhsT=wx_all[
                                    :, m * (DR + 2 * DS) : (m + 1) * (DR + 2 * DS)
                                ],
                                rhs=xcb[:, c * CH : (c + 1) * CH],
                                start=(m == 0),
                                stop=(m == ND - 1),
                            )
                        nc.sync.dma_start(xcb_scr[:, m * L : (m + 1) * L], xcb[:])
                    for c in range(NCH):
                        nc.scalar.activation(
                            dbc_bf[:, c * CH : (c + 1) * CH], psx[c][:], AF.Copy
                        )

                # ---- stage 2: broadcast B and C rows via replicated DMA ----
                nc.sync.dma_start(bc_scr[:], dbc_bf[DR : DR + 2 * DS, :])
                for s in range(DS):
                    nc.sync.dma_start(
                        Bc[:, s * L : (s + 1) * L],
                        bc_scr[s : s + 1, :].broadcast_to([P, L]),
                    )
                    nc.sync.dma_start(
                        Cc[:, s * L : (s + 1) * L],
                        bc_scr[DS + s : DS + s + 1, :].broadcast_to([P, L]),
                    )

                # ---- stage 3: z-half + dt head + scan + gate ---------------
                g_all = [None] * 4
                with tc.tile_pool(name="s3", bufs=2) as s3p, \
                     tc.tile_pool(name="s3g", bufs=4) as s3g, \
                     tc.tile_pool(name="s3a", bufs=2) as s3a, \
                     tc.tile_pool(name="s3b", bufs=2) as s3b, \
                     tc.tile_pool(name="s3h", bufs=1) as s3h, \
                     tc.tile_pool(name="s3m", bufs=1) as s3m, \
                     tc.tile_pool(name="s4w", bufs=2) as s4wp, \
                     tc.tile_pool(name="s4ho", bufs=2) as s4hop, \
                     tc.tile_pool(name="psd", bufs=1, space="PSUM") as psdp, \
                     tc.tile_pool(name="psz", bufs=2, space="PSUM") as pszp, \
                     tc.tile_pool(name="psh1", bufs=1, space="PSUM") as psh1p, \
                     tc.tile_pool(name="psy", bufs=2, space="PSUM") as psyp:
                    for m in range(ND):
                        # z-half matmuls + silu batched in quads of m so the
                        # silu<->exp/ln ACT-table switch happens 2x per 4 m
                        if m % 2 == 0:
                            for m2 in range(m, m + 2):
                                g_all[m2 % 4] = s3g.tile(
                                    [P, L], BF, tag="gm", name=f"g{m2}"
                                )
                                wiz = wip.tile(
                                    [P, NK_D // 2, 2, P], xdt, tag="wi", name=f"wiz{m2}"
                                )
                                nc.sync.dma_start(wiz[:], w_in[ND + m2])
                                for c in range(NCH):
                                    pz = pszp.tile([P, CH], F32, tag="psz")
                                    if IN_FP8:
                                        for kp in range(NK_D // 2):
                                            nc.tensor.matmul(
                                                pz[:],
                                                lhsT=wiz[:, kp],
                                                rhs=kx[:, 2 * kp : 2 * kp + 2, c * CH : (c + 1) * CH],
                                                start=(kp == 0),
                                                stop=(kp == NK_D // 2 - 1),
                                                perf_mode=mybir.MatmulPerfMode.DoubleRow,
                                            )
                                    else:
                                        for k in range(NK_D):
                                            nc.tensor.matmul(
                                                pz[:],
                                                lhsT=wiz[:, k // 2, k % 2],
                                                rhs=kx[:, k, c * CH : (c + 1) * CH],
                                                start=(k == 0),
                                                stop=(k == NK_D - 1),
                                            )
                                    nc.scalar.activation(
                                        g_all[m2 % 4][:, c * CH : (c + 1) * CH],
                                        pz[:], AF.Silu,
                                        scale=(1.0 / W_SCALE) if IN_FP8 else 1.0,
                                    )
                        g_m = g_all[m % 4]

                        xcb_m = s3p.tile([P, L], BF, tag="xcbm")
                        nc.sync.dma_start(xcb_m[:], xcb_scr[:, m * L : (m + 1) * L])

                        e_t = s3p.tile([P, L], F32, tag="e")
                        for c in range(NCH):
                            psd = psdp.tile([P, CH], F32, tag="psd", name=f"psd{m}_{c}")
                            nc.tensor.matmul(
                                psd[:],
                                lhsT=wdt_all[:, m * P : (m + 1) * P],
                                rhs=dbc_bf[0:DR, c * CH : (c + 1) * CH],
                                start=True,
                                stop=True,
                            )
                            nc.scalar.activation(
                                e_t[:, c * CH : (c + 1) * CH], psd[:], AF.Exp,
                                bias=cc(m, 5),
                            )
                        delta = s3p.tile([P, L], F32, tag="delta")
                        nc.scalar.activation(delta[:], e_t[:], AF.Ln, bias=1.0)
                        delta_b = s3p.tile([P, L], BF, tag="deltab")
                        nc.scalar.activation(delta_b[:], delta[:], AF.Copy)
                        du = s3p.tile([P, L], BF, tag="du")
                        nc.vector.tensor_mul(du[:], delta_b[:], xcb_m[:])

                        mD = s3p.tile([P, L], BF, tag="mD")
                        nc.scalar.activation(mD[:], xcb_m[:], AF.Copy, scale=cc(m, 6))
                        psy = psyp.tile([P, L], F32, tag="psy")
                        for c in range(NCH):
                            nc.tensor.matmul(
                                psy[:, c * CH : (c + 1) * CH],
                                lhsT=ident[:],
                                rhs=mD[:, c * CH : (c + 1) * CH],
                                start=True,
                                stop=False,
                            )
                        for q in range(NG):
                            s0 = q * G
                            agrp = s3a.tile([P, G * L], BF, tag="agrp")
                            for j in range(G):
                                nc.scalar.activation(
                                    agrp[:, j * L : (j + 1) * L], delta[:], AF.Exp,
                                    scale=cc(m, 7 + s0 + j),
                                )
                            bgrp = s3b.tile([P, G * L], BF, tag="bgrp")
                            nc.vector.tensor_mul(
                                bgrp[:],
                                du[:, None, :].broadcast_to([P, G, L]),
                                Bc[:, s0 * L : (s0 + G) * L],
                            )
                            hgrp = s3h.tile([P, G * L], BF, tag="hgrp")
                            for j in range(G):
                                nc.vector.tensor_tensor_scan(
                                    hgrp[:, j * L : (j + 1) * L],
                                    agrp[:, j * L : (j + 1) * L],
                                    bgrp[:, j * L : (j + 1) * L],
                                    0.0, op0=OP.mult, op1=OP.add,
                                )
                            mgrp = s3m.tile([P, G * L], BF, tag="mgrp")
                            nc.vector.tensor_mul(
                                mgrp[:], hgrp[:], Cc[:, s0 * L : (s0 + G) * L]
                            )
                            for j in range(G):
                                for c in range(NCH):
                                    nc.tensor.matmul(
                                        psy[:, c * CH : (c + 1) * CH],
                                        lhsT=ident[:],
                                        rhs=mgrp[:, j * L + c * CH : j * L + (c + 1) * CH],
                                        start=False,
                                        stop=(q == NG - 1 and j == G - 1),
                                    )
                        yb16 = s3p.tile([P, L], BF, tag="yb16")
                        nc.scalar.activation(yb16[:], psy[:], AF.Copy, scale=Y_SCALE)
                        og16 = s3p.tile([P, L], BF, tag="og16")
                        nc.vector.tensor_mul(og16[:], yb16[:], g_m[:])
                        nc.scalar.activation(opre[:, m, :], og16[:], AF.Copy)
                        if m >= NN:
                            # first k-half of out_proj for n = m - NN
                            n = m - NN
                            wo = s4wp.tile([P, ND // 2, 2, P], F8, tag="wo",
                                           name=f"wo{n}")
                            nc.sync.dma_start(wo[:], w_out[n])
                            oh1 = s4hop.tile([P, L], BF, tag="oh1", name=f"oh1_{n}")
                            for c in range(NCH):
                                ph = psh1p.tile([P, CH], F32, tag="ph",
                                                name=f"ph{n}_{c}")
                                for kp in range(ND // 4):
                                    nc.tensor.matmul(
                                        ph[:],
                                        lhsT=wo[:, kp],
                                        rhs=opre[:, 2 * kp : 2 * kp + 2,
                                                 c * CH : (c + 1) * CH],
                                        start=(kp == 0),
                                        stop=(kp == ND // 4 - 1),
                                        perf_mode=mybir.MatmulPerfMode.DoubleRow,
                                    )
                                nc.scalar.activation(
                                    oh1[:, c * CH : (c + 1) * CH], ph[:], AF.Copy,
                                    scale=1.0 / (W_SCALE * Y_SCALE),
                                )
                            nc.sync.dma_start(oh1_scr[:, n, :], oh1[:])

            # ------- stage 4: out_proj second k-half + merge with oh1 -------
            with tc.tile_pool(name="s4r", bufs=4) as s4r, \
                 tc.tile_pool(name="s4o", bufs=4) as s4o, \
                 tc.tile_pool(name="pso", bufs=4, space="PSUM") as psop:
                for n in range(NN):
                    wo = s4r.tile([P, ND // 2, 2, P], F8, tag="wo2", name=f"wo2_{n}")
                    nc.sync.dma_start(wo[:], w_out[n])
                    oh1r = s4r.tile([P, L], BF, tag="oh1r", name=f"oh1r{n}")
                    nc.sync.dma_start(oh1r[:], oh1_scr[:, n, :])
                    for c in range(NCH):
                        pso = psop.tile([P, CH], F32, tag="pso")
                        for kp in range(ND // 4, ND // 2):
                            nc.tensor.matmul(
                                pso[:],
                                lhsT=wo[:, kp],
                                rhs=opre[:, 2 * kp : 2 * kp + 2, c * CH : (c + 1) * CH],
                                start=(kp == ND // 4),
                                stop=(kp == ND // 2 - 1),
                                perf_mode=mybir.MatmulPerfMode.DoubleRow,
                            )
                        ob = s4o.tile([P, CH], F32, tag="ob")
                        nc.vector.scalar_tensor_tensor(
                            out=ob[:], in0=pso[:],
                            scalar=1.0 / (W_SCALE * Y_SCALE),
                            in1=oh1r[:, c * CH : (c + 1) * CH],
                            op0=OP.mult, op1=OP.add,
                        )
                        nc.sync.dma_start(
                            out[n * P : (n + 1) * P, c * CH : (c + 1) * CH], ob[:]
                        )

    split_excess_waits(nc)
    return nc


_NC = None


def _get_nc():
    global _NC
    if _NC is None:
        _NC = _build_program()
    return _NC


def _prep_core(x_b, flip, in_proj, conv_w, conv_b, x_proj, dt_w, dt_b, A_log, Dsk, out_proj):
    xdt_np = FP8 if IN_FP8 else BF16
    xtr = x_b[::-1].T if flip else x_b.T  # [D, L] fp32
    xt = np.ascontiguousarray(
        xtr.astype(xdt_np).reshape(NK_D, P, L).transpose(1, 0, 2)
    ).reshape(P, NK_D * L)

    w_in_f = in_proj.T.astype(np.float32)
    if IN_FP8:
        w_in_f = w_in_f * W_SCALE
    w_in_t = w_in_f.astype(xdt_np)  # [D, 2DI]
    # [k, P, 2ND, P] -> [2ND, P, kp, j, P] with k = 2*kp + j
    w_in = np.ascontiguousarray(
        w_in_t.reshape(NK_D // 2, 2, P, 2 * ND, P).transpose(3, 2, 0, 1, 4)
    )

    w_x_t = x_proj.T.astype(BF16)  # [DI, 96]
    w_x = np.ascontiguousarray(
        w_x_t.reshape(ND, P, DR + 2 * DS).transpose(1, 0, 2)
    ).reshape(P, ND * (DR + 2 * DS))

    w_dt = np.ascontiguousarray(dt_w.T.astype(BF16)).reshape(DR, ND * P)

    w_out_t = (out_proj.T.astype(np.float32) * W_SCALE).astype(FP8)  # [DI, D]
    w_out = np.ascontiguousarray(
        w_out_t.reshape(ND // 2, 2, P, NN, P).transpose(3, 2, 0, 1, 4)
    )

    A = -np.exp(A_log.astype(np.float64)).astype(np.float32)  # [DI, DS]
    chan_flat = np.concatenate(
        [
            conv_w.astype(np.float32),
            conv_b[:, None].astype(np.float32),
            dt_b[:, None].astype(np.float32),
            Dsk[:, None].astype(np.float32),
            A,
        ],
        axis=1,
    )
    chan = np.ascontiguousarray(
        chan_flat.reshape(ND, P, NCOLS).transpose(1, 0, 2)
    ).reshape(P, ND * NCOLS)

    return {
        "xt": xt,
        "w_in": w_in,
        "w_x": w_x,
        "w_dt": w_dt,
        "w_out": w_out,
        "chan": chan,
    }


def kernel(**inputs):
    global LAST_EXEC_NS, LAST_RESULTS
    inputs = {k: np.asarray(v) for k, v in inputs.items()}
    x = inputs["x"]

    in_maps = []
    for i in range(8):
        b = i % B
        p = "f" if i < B else "b"
        in_maps.append(
            _prep_core(
                x[b],
                flip=(p == "b"),
                in_proj=inputs[f"in_proj_{p}"],
                conv_w=inputs[f"conv_w_{p}"],
                conv_b=inputs[f"conv_b_{p}"],
                x_proj=inputs[f"x_proj_{p}"],
                dt_w=inputs[f"dt_w_{p}"],
                dt_b=inputs[f"dt_b_{p}"],
                A_log=inputs[f"A_log_{p}"],
                Dsk=inputs[f"D_{p}"],
                out_proj=inputs[f"out_proj_{p}"],
            )
        )

    trace = bool(os.environ.get("MAMBA_TRACE"))
    if trace:
        _install_ntff_hook()
    nc = _get_nc()
    res = run_bass_kernel_spmd(nc, in_maps, core_ids=list(range(8)), trace=trace)
    LAST_EXEC_NS = res.exec_time_ns
    LAST_RESULTS = res

    h = x.astype(np.float32).copy()
    for i in range(8):
        y = res.results[i]["out"].T  # [L, D]
        if i >= B:
            y = y[::-1]
        h[i % B] += y
    mu = h.mean(axis=-1, keepdims=True, dtype=np.float64)
    var = np.mean((h - mu) ** 2, axis=-1, keepdims=True, dtype=np.float64)
    outp = (h - mu) / np.sqrt(var + 1e-5) * inputs["ln_w"] + inputs["ln_b"]
    return outp.astype(np.float32)


# revision 3
# speedup vs baseline: 1.0080x; 1.0080x over previous
"""Bidirectional Mamba block (B=4, L=1024, D=1024, DI=2048, DS=16) on 8
Trainium2 NeuronCores.

Sharding: one (batch, direction) pair per core - 8 fully data-parallel
shards, no collectives. Host flips the backward direction, sums the
residual and applies the final LayerNorm while gathering.

Per-core schedule (DVE-saturated; scans at the hardware 2cyc/elem floor):
- stage 1: in_proj xh-half in fp8e4 DoubleRow (2x PE) + causal conv on
  DVE (fills the otherwise-idle pre-scan window) + silu + x_proj
- stage 3: z-half (fp8 DR) + silu gate batched in pairs of d-tiles (2
  ACT-table loads per pair), dt head, softplus via exp/ln (table 6),
  16 decay exps per d-tile on ACT, b/m elementwise muls batched 4
  states per wide 2x DVE op (broadcast-stride-0 du), selective scan on
  DVE, state-sum via identity-matmul PSUM accumulation on PE, gate
- out_proj in fp8e4 DoubleRow: first k-half woven under stage 3
  (partials spilled to DRAM); second half + merge software-pipelined
  with weight/partial reload DMAs prefetched 3 iterations ahead
- gpsimd stays idle during stage 3: it shares an SBUF port pair with
  the DVE and measurably slows the scans when active
"""

import os
import sys
import types

sys.path.insert(0, "/opt/trn_rl_repo")

import numpy as np
import ml_dtypes

BF16 = ml_dtypes.bfloat16
FP8 = ml_dtypes.float8_e4m3

import concourse.bass as bass
import concourse.mybir as mybir
from concourse.tile import TileContext
from concourse.bass_utils import run_bass_kernel_spmd
from concourse.masks import make_identity

P = 128
B, L, D = 4, 1024, 1024
DI, DS, DC, DR = 2048, 16, 4, 64
ND = DI // P          # 16 d-tiles
NK_D = D // P         # 8 k-tiles over D
NN = D // P           # 8 n-tiles of out_proj output
CH = 512              # psum chunk (free dim)
NCH = L // CH
NCOLS = 7 + DS        # conv_w(4), conv_b, dt_b, D, A(16)
G = 4                 # states per mul-group
NG = DS // G          # 4 groups

# engine split: which s-groups' muls run on gpsimd (rest on DVE, batched)
B_GP_GROUPS = ()          # gpsimd mul offload disabled (hurts DVE)
M_GP_GROUPS = ()

IN_FP8 = True         # in_proj (both halves) in fp8e4 DoubleRow
W_SCALE = 64.0        # fp8 weight scale
Y_SCALE = 8.0         # pre-gate y scale (fp8 opre dynamic range)

F32 = mybir.dt.float32
BF = mybir.dt.bfloat16
F8 = mybir.dt.float8e4
AF = mybir.ActivationFunctionType
OP = mybir.AluOpType

LAST_EXEC_NS = None
LAST_RESULTS = None


def _install_ntff_hook():
    import antenv

    if "antenv.axon_hooks" in sys.modules:
        return
    mod = types.ModuleType("antenv.axon_hooks")
    mod._hook = None
    mod.set_axon_ntff_profile_hook = lambda h: setattr(mod, "_hook", h)
    mod.get_axon_ntff_profile_hook = lambda: mod._hook
    sys.modules["antenv.axon_hooks"] = mod
    antenv.axon_hooks = mod
    try:
        from trn_agent_boot.trn_boot import _ntff_profile_via_ctypes

        mod.set_axon_ntff_profile_hook(
            _ntff_profile_via_ctypes("/opt/axon/libaxon_pjrt.so")
        )
    except Exception:
        pass


def split_excess_waits(nc, max_waits=1):
    """Walrus encodes at most `max_waits` sync-wait commands per instruction."""
    n_extra = 0
    for f in nc.m.functions:
        for bb in f.blocks:
            insts = bb.instructions
            i = 0
            while i < len(insts):
                inst = insts[i]
                si = inst.sync_info
                if si is not None and len(si.on_wait) > max_waits:
                    waits = list(si.on_wait)
                    for j, w in enumerate(waits[max_waits:]):
                        nop = mybir.InstNoOp(
                            name=f"{inst.name}-xw{j}",
                            engine=inst.engine,
                            bass_nofuse=True,
                            sync_info=mybir.SyncInfo(on_wait=[w], on_update=[]),
                        )
                        insts.insert(i, nop)
                        i += 1
                        n_extra += 1
                    inst.sync_info = mybir.SyncInfo(
                        on_wait=waits[:max_waits], on_update=list(si.on_update)
                    )
                i += 1
    return n_extra


def _build_program():
    nc = bass.Bass("TRN2")

    xdt = F8 if IN_FP8 else BF
    xt = nc.dram_tensor("xt", [P, NK_D * L], xdt, kind="ExternalInput")
    w_in = nc.dram_tensor("w_in", [2 * ND, P, NK_D // 2, 2, P], xdt, kind="ExternalInput")
    w_x = nc.dram_tensor("w_x", [P, ND * (DR + 2 * DS)], BF, kind="ExternalInput")
    w_dt = nc.dram_tensor("w_dt", [DR, ND * P], BF, kind="ExternalInput")
    w_out = nc.dram_tensor("w_out", [NN, P, ND // 2, 2, P], F8, kind="ExternalInput")
    chan = nc.dram_tensor("chan", [P, ND * NCOLS], F32, kind="ExternalInput")
    out = nc.dram_tensor("out", [D, L], F32, kind="ExternalOutput")

    xcb_scr = nc.dram_tensor("xcb_scr", [P, ND * L], BF)
    oh1_scr = nc.dram_tensor("oh1_scr", [P, NN, L], BF)
    bc_scr = nc.dram_tensor("bc_scr", [2 * DS, L], BF)

    with TileContext(nc) as tc:
        with tc.tile_pool(name="res", bufs=1) as res:
            Bc = res.tile([P, DS * L], BF, tag="Bc")
            Cc = res.tile([P, DS * L], BF, tag="Cc")
            ident = res.tile([P, P], BF, tag="ident")
            dbc_bf = res.tile([DR + 2 * DS, L], BF, tag="dbcbf")
            chan_all = res.tile([P, ND * NCOLS], F32, tag="chan")
            wx_all = res.tile([P, ND * (DR + 2 * DS)], BF, tag="wx")
            wdt_all = res.tile([DR, ND * P], BF, tag="wdt")
            kx = res.tile([P, NK_D, L], xdt, tag="kx")
            opre = res.tile([P, ND, L], F8, tag="opre")

            make_identity(nc, ident[:])
            nc.sync.dma_start(chan_all[:], chan[:])
            nc.sync.dma_start(wx_all[:], w_x[:])
            nc.sync.dma_start(wdt_all[:], w_dt[:])
            for k in range(NK_D):
                nc.sync.dma_start(
                    kx[:, k, :], xt[:, k * L : (k + 1) * L]
                )

            def cc(m, col):
                return chan_all[:, m * NCOLS + col : m * NCOLS + col + 1]

            with tc.tile_pool(name="wi", bufs=3) as wip:
                # ---- stage 1: in_proj xh-half + conv(DVE) + silu + x_proj --
                with tc.tile_pool(name="s1", bufs=4) as s1p, \
                     tc.tile_pool(name="s1b", bufs=2) as s1q, \
                     tc.tile_pool(name="psh", bufs=3, space="PSUM") as pshp, \
                     tc.tile_pool(name="ps2", bufs=2, space="PSUM") as ps2p:
                    psx = [
                        ps2p.tile([DR + 2 * DS, CH], F32, tag="psx", name=f"psx{c}")
                        for c in range(NCH)
                    ]
                    for m in range(ND):
                        xh = s1q.tile([P, 3 + L], BF, tag="xh")
                        nc.gpsimd.memset(xh[:, 0:3], 0.0)
                        wi = wip.tile([P, NK_D // 2, 2, P], xdt, tag="wi", name=f"wia{m}")
                        nc.sync.dma_start(wi[:], w_in[m])
                        for c in range(NCH):
                            ps = pshp.tile([P, CH], F32, tag="psh")
                            if IN_FP8:
                                for kp in range(NK_D // 2):
                                    nc.tensor.matmul(
                                        ps[:],
                                        lhsT=wi[:, kp],
                                        rhs=kx[:, 2 * kp : 2 * kp + 2, c * CH : (c + 1) * CH],
                                        start=(kp == 0),
                                        stop=(kp == NK_D // 2 - 1),
                                        perf_mode=mybir.MatmulPerfMode.DoubleRow,
                                    )
                            else:
                                for k in range(NK_D):
                                    nc.tensor.matmul(
                                        ps[:],
                                        lhsT=wi[:, k // 2, k % 2],
                                        rhs=kx[:, k, c * CH : (c + 1) * CH],
                                        start=(k == 0),
                                        stop=(k == NK_D - 1),
                                    )
                            nc.scalar.activation(
                                xh[:, 3 + c * CH : 3 + (c + 1) * CH], ps[:], AF.Copy,
                                scale=(1.0 / W_SCALE) if IN_FP8 else 1.0,
                            )
                        # causal conv on DVE: acc = xh0*w0 + conv_b, 3 fused FMAs
                        acc0 = s1p.tile([P, L], F32, tag="accmid", name=f"ac0_{m}")
                        nc.vector.tensor_scalar(
                            out=acc0[:], in0=xh[:, 0:L],
                            scalar1=cc(m, 0), scalar2=cc(m, 4),
                            op0=OP.mult, op1=OP.add,
                        )
                        acc1 = s1p.tile([P, L], F32, tag="accmid", name=f"ac1_{m}")
                        nc.vector.scalar_tensor_tensor(
                            out=acc1[:], in0=xh[:, 1 : 1 + L], scalar=cc(m, 1),
                            in1=acc0[:], op0=OP.mult, op1=OP.add,
                        )
                        acc2 = s1p.tile([P, L], F32, tag="accmid", name=f"ac2_{m}")
                        nc.vector.scalar_tensor_tensor(
                            out=acc2[:], in0=xh[:, 2 : 2 + L], scalar=cc(m, 2),
                            in1=acc1[:], op0=OP.mult, op1=OP.add,
                        )
                        acc3 = s1p.tile([P, L], F32, tag="acc3", name=f"ac3_{m}")
                        nc.vector.scalar_tensor_tensor(
                            out=acc3[:], in0=xh[:, 3 : 3 + L], scalar=cc(m, 3),
                            in1=acc2[:], op0=OP.mult, op1=OP.add,
                        )
                        xcb = s1q.tile([P, L], BF, tag="xcb")
                        for c in range(NCH):
                            nc.scalar.activation(
                                xcb[:, c * CH : (c + 1) * CH],
                                acc3[:, c * CH : (c + 1) * CH], AF.Silu,
                            )
                            nc.tensor.matmul(
                                psx[c][:],
                                lhsT=wx_all[
                                    :, m * (DR + 2 * DS) : (m + 1) * (DR + 2 * DS)
                                ],
                                rhs=xcb[:, c * CH : (c + 1) * CH],
                                start=(m == 0),
                                stop=(m == ND - 1),
                            )
                        nc.sync.dma_start(xcb_scr[:, m * L : (m + 1) * L], xcb[:])
                    for c in range(NCH):
                        nc.scalar.activation(
                            dbc_bf[:, c * CH : (c + 1) * CH], psx[c][:], AF.Copy
                        )

                # ---- stage 2: broadcast B and C rows via replicated DMA ----
                nc.sync.dma_start(bc_scr[:], dbc_bf[DR : DR + 2 * DS, :])
                for s in range(DS):
                    nc.sync.dma_start(
                        Bc[:, s * L : (s + 1) * L],
                        bc_scr[s : s + 1, :].broadcast_to([P, L]),
                    )
                    nc.sync.dma_start(
                        Cc[:, s * L : (s + 1) * L],
                        bc_scr[DS + s : DS + s + 1, :].broadcast_to([P, L]),
                    )

                # ---- stage 3: z-half + dt head + scan + gate ---------------
                g_all = [None] * 4
                with tc.tile_pool(name="s3", bufs=2) as s3p, \
                     tc.tile_pool(name="s3g", bufs=4) as s3g, \
                     tc.tile_pool(name="s3a", bufs=2) as s3a, \
                     tc.tile_pool(name="s3b", bufs=2) as s3b, \
                     tc.tile_pool(name="s3h", bufs=1) as s3h, \
                     tc.tile_pool(name="s3m", bufs=1) as s3m, \
                     tc.tile_pool(name="s4w", bufs=2) as s4wp, \
                     tc.tile_pool(name="s4ho", bufs=2) as s4hop, \
                     tc.tile_pool(name="psd", bufs=1, space="PSUM") as psdp, \
                     tc.tile_pool(name="psz", bufs=2, space="PSUM") as pszp, \
                     tc.tile_pool(name="psh1", bufs=1, space="PSUM") as psh1p, \
                     tc.tile_pool(name="psy", bufs=2, space="PSUM") as psyp:
                    for m in range(ND):
                        # z-half matmuls + silu batched in quads of m so the
                        # silu<->exp/ln ACT-table switch happens 2x per 4 m
                        if m % 2 == 0:
                            for m2 in range(m, m + 2):
                                g_all[m2 % 4] = s3g.tile(
                                    [P, L], BF, tag="gm", name=f"g{m2}"
                                )
                                wiz = wip.tile(
                                    [P, NK_D // 2, 2, P], xdt, tag="wi", name=f"wiz{m2}"
                                )
                                nc.sync.dma_start(wiz[:], w_in[ND + m2])
                                for c in range(NCH):
                                    pz = pszp.tile([P, CH], F32, tag="psz")
                                    if IN_FP8:
                                        for kp in range(NK_D // 2):
                                            nc.tensor.matmul(
                                                pz[:],
                                                lhsT=wiz[:, kp],
                                                rhs=kx[:, 2 * kp : 2 * kp + 2, c * CH : (c + 1) * CH],
                                                start=(kp == 0),
                                                stop=(kp == NK_D // 2 - 1),
                                                perf_mode=mybir.MatmulPerfMode.DoubleRow,
                                            )
                                    else:
                                        for k in range(NK_D):
                                            nc.tensor.matmul(
                                                pz[:],
                                                lhsT=wiz[:, k // 2, k % 2],
                                                rhs=kx[:, k, c * CH : (c + 1) * CH],
                                                start=(k == 0),
                                                stop=(k == NK_D - 1),
                                            )
                                    nc.scalar.activation(
                                        g_all[m2 % 4][:, c * CH : (c + 1) * CH],
                                        pz[:], AF.Silu,
                                        scale=(1.0 / W_SCALE) if IN_FP8 else 1.0,
                                    )
                        g_m = g_all[m % 4]

                        xcb_m = s3p.tile([P, L], BF, tag="xcbm")
                        nc.sync.dma_start(xcb_m[:], xcb_scr[:, m * L : (m + 1) * L])

                        e_t = s3p.tile([P, L], F32, tag="e")
                        for c in range(NCH):
                            psd = psdp.tile([P, CH], F32, tag="psd", name=f"psd{m}_{c}")
                            nc.tensor.matmul(
                                psd[:],
                                lhsT=wdt_all[:, m * P : (m + 1) * P],
                                rhs=dbc_bf[0:DR, c * CH : (c + 1) * CH],
                                start=True,
                                stop=True,
                            )
                            nc.scalar.activation(
                                e_t[:, c * CH : (c + 1) * CH], psd[:], AF.Exp,
                                bias=cc(m, 5),
                            )
                        delta = s3p.tile([P, L], F32, tag="delta")
                        nc.scalar.activation(delta[:], e_t[:], AF.Ln, bias=1.0)
                        delta_b = s3p.tile([P, L], BF, tag="deltab")
                        nc.scalar.activation(delta_b[:], delta[:], AF.Copy)
                        du = s3p.tile([P, L], BF, tag="du")
                        nc.vector.tensor_mul(du[:], delta_b[:], xcb_m[:])

                        mD = s3p.tile([P, L], BF, tag="mD")
                        nc.scalar.activation(mD[:], xcb_m[:], AF.Copy, scale=cc(m, 6))
                        psy = psyp.tile([P, L], F32, tag="psy")
                        for c in range(NCH):
                            nc.tensor.matmul(
                                psy[:, c * CH : (c + 1) * CH],
                                lhsT=ident[:],
                                rhs=mD[:, c * CH : (c + 1) * CH],
                                start=True,
                                stop=False,
                            )
                        for q in range(NG):
                            s0 = q * G
                            agrp = s3a.tile([P, G * L], BF, tag="agrp")
                            for j in range(G):
                                nc.scalar.activation(
                                    agrp[:, j * L : (j + 1) * L], delta[:], AF.Exp,
                                    scale=cc(m, 7 + s0 + j),
                                )
                            bgrp = s3b.tile([P, G * L], BF, tag="bgrp")
                            nc.vector.tensor_mul(
                                bgrp[:],
                                du[:, None, :].broadcast_to([P, G, L]),
                                Bc[:, s0 * L : (s0 + G) * L],
                            )
                            hgrp = s3h.tile([P, G * L], BF, tag="hgrp")
                            for j in range(G):
                                nc.vector.tensor_tensor_scan(
                                    hgrp[:, j * L : (j + 1) * L],
                                    agrp[:, j * L : (j + 1) * L],
                                    bgrp[:, j * L : (j + 1) * L],
                                    0.0, op0=OP.mult, op1=OP.add,
                                )
                            mgrp = s3m.tile([P, G * L], BF, tag="mgrp")
                            nc.vector.tensor_mul(
                                mgrp[:], hgrp[:], Cc[:, s0 * L : (s0 + G) * L]
                            )
                            for j in range(G):
                                for c in range(NCH):
                                    nc.tensor.matmul(
                                        psy[:, c * CH : (c + 1) * CH],
                                        lhsT=ident[:],
                                        rhs=mgrp[:, j * L + c * CH : j * L + (c + 1) * CH],
                                        start=False,
                                        stop=(q == NG - 1 and j == G - 1),
                                    )
                        yb16 = s3p.tile([P, L], BF, tag="yb16")
                        nc.scalar.activation(yb16[:], psy[:], AF.Copy, scale=Y_SCALE)
                        og16 = s3p.tile([P, L], BF, tag="og16")
                        nc.vector.tensor_mul(og16[:], yb16[:], g_m[:])
                        nc.scalar.activation(opre[:, m, :], og16[:], AF.Copy)
                        if m >= NN:
                            # first k-half of out_proj for n = m - NN
                            n = m - NN
                            wo = s4wp.tile([P, ND // 2, 2, P], F8, tag="wo",
                                           name=f"wo{n}")
                            nc.sync.dma_start(wo[:], w_out[n])
                            oh1 = s4hop.tile([P, L], BF, tag="oh1", name=f"oh1_{n}")
                            for c in range(NCH):
                                ph = psh1p.tile([P, CH], F32, tag="ph",
                                                name=f"ph{n}_{c}")
                                for kp in range(ND // 4):
                                    nc.tensor.matmul(
                                        ph[:],
                                        lhsT=wo[:, kp],
                                        rhs=opre[:, 2 * kp : 2 * kp + 2,
                                                 c * CH : (c + 1) * CH],
                                        start=(kp == 0),
                                        stop=(kp == ND // 4 - 1),
                                        perf_mode=mybir.MatmulPerfMode.DoubleRow,
                                    )
                                nc.scalar.activation(
                                    oh1[:, c * CH : (c + 1) * CH], ph[:], AF.Copy,
                                    scale=1.0 / (W_SCALE * Y_SCALE),
                                )
                            nc.sync.dma_start(oh1_scr[:, n, :], oh1[:])

            # ------- stage 4: out_proj second k-half + merge with oh1 -------
            # software-pipelined: wo/oh1r DMAs issued 3 iterations ahead so
            # the merge chains never wait on the sync queue
            with tc.tile_pool(name="s4r", bufs=4) as s4r, \
                 tc.tile_pool(name="s4w2", bufs=4) as s4w2, \
                 tc.tile_pool(name="s4o", bufs=4) as s4o, \
                 tc.tile_pool(name="pso", bufs=4, space="PSUM") as psop:
                wos2 = [None] * NN
                oh1rs = [None] * NN
                PF = 3

                def _fetch(n):
                    wos2[n] = s4w2.tile([P, ND // 2, 2, P], F8, tag="wo2",
                                        name=f"wo2_{n}")
                    nc.sync.dma_start(wos2[n][:], w_out[n])
                    oh1rs[n] = s4r.tile([P, L], BF, tag="oh1r", name=f"oh1r{n}")
                    nc.sync.dma_start(oh1rs[n][:], oh1_scr[:, n, :])

                for n in range(min(PF, NN)):
                    _fetch(n)
                for n in range(NN):
                    if n + PF < NN:
                        _fetch(n + PF)
                    wo = wos2[n]
                    oh1r = oh1rs[n]
                    for c in range(NCH):
                        pso = psop.tile([P, CH], F32, tag="pso")
                        for kp in range(ND // 4, ND // 2):
                            nc.tensor.matmul(
                                pso[:],
                                lhsT=wo[:, kp],
                                rhs=opre[:, 2 * kp : 2 * kp + 2, c * CH : (c + 1) * CH],
                                start=(kp == ND // 4),
                                stop=(kp == ND // 2 - 1),
                                perf_mode=mybir.MatmulPerfMode.DoubleRow,
                            )
                        ob = s4o.tile([P, CH], F32, tag="ob")
                        nc.vector.scalar_tensor_tensor(
                            out=ob[:], in0=pso[:],
                            scalar=1.0 / (W_SCALE * Y_SCALE),
                            in1=oh1r[:, c * CH : (c + 1) * CH],
                            op0=OP.mult, op1=OP.add,
                        )
                        nc.sync.dma_start(
                            out[n * P : (n + 1) * P, c * CH : (c + 1) * CH], ob[:]
                        )

    split_excess_waits(nc)
    return nc


_NC = None


def _get_nc():
    global _NC
    if _NC is None:
        _NC = _build_program()
    return _NC


def _prep_core(x_b, flip, in_proj, conv_w, conv_b, x_proj, dt_w, dt_b, A_log, Dsk, out_proj):
    xdt_np = FP8 if IN_FP8 else BF16
    xtr = x_b[::-1].T if flip else x_b.T  # [D, L] fp32
    xt = np.ascontiguousarray(
        xtr.astype(xdt_np).reshape(NK_D, P, L).transpose(1, 0, 2)
    ).reshape(P, NK_D * L)

    w_in_f = in_proj.T.astype(np.float32)
    if IN_FP8:
        w_in_f = w_in_f * W_SCALE
    w_in_t = w_in_f.astype(xdt_np)  # [D, 2DI]
    # [k, P, 2ND, P] -> [2ND, P, kp, j, P] with k = 2*kp + j
    w_in = np.ascontiguousarray(
        w_in_t.reshape(NK_D // 2, 2, P, 2 * ND, P).transpose(3, 2, 0, 1, 4)
    )

    w_x_t = x_proj.T.astype(BF16)  # [DI, 96]
    w_x = np.ascontiguousarray(
        w_x_t.reshape(ND, P, DR + 2 * DS).transpose(1, 0, 2)
    ).reshape(P, ND * (DR + 2 * DS))

    w_dt = np.ascontiguousarray(dt_w.T.astype(BF16)).reshape(DR, ND * P)

    w_out_t = (out_proj.T.astype(np.float32) * W_SCALE).astype(FP8)  # [DI, D]
    w_out = np.ascontiguousarray(
        w_out_t.reshape(ND // 2, 2, P, NN, P).transpose(3, 2, 0, 1, 4)
    )

    A = -np.exp(A_log.astype(np.float64)).astype(np.float32)  # [DI, DS]
    chan_flat = np.concatenate(
        [
            conv_w.astype(np.float32),
            conv_b[:, None].astype(np.float32),
            dt_b[:, None].astype(np.float32),
            Dsk[:, None].astype(np.float32),
            A,
        ],
        axis=1,
    )
    chan = np.ascontiguousarray(
        chan_flat.reshape(ND, P, NCOLS).transpose(1, 0, 2)
    ).reshape(P, ND * NCOLS)

    return {
        "xt": xt,
        "w_in": w_in,
        "w_x": w_x,
        "w_dt": w_dt,
        "w_out": w_out,
        "chan": chan,
    }


def kernel(**inputs):
    global LAST_EXEC_NS, LAST_RESULTS
    inputs = {k: np.asarray(v) for k, v in inputs.items()}
    x = inputs["x"]

    in_maps = []
    for i in range(8):
        b = i % B
        p = "f" if i < B else "b"
        in_maps.append(
            _prep_core(
                x[b],
                flip=(p == "b"),
                in_proj=inputs[f"in_proj_{p}"],
                conv_w=inputs[f"conv_w_{p}"],
                conv_b=inputs[f"conv_b_{p}"],
                x_proj=inputs[f"x_proj_{p}"],
                dt_w=inputs[f"dt_w_{p}"],
                dt_b=inputs[f"dt_b_{p}"],
                A_log=inputs[f"A_log_{p}"],
                Dsk=inputs[f"D_{p}"],
                out_proj=inputs[f"out_proj_{p}"],
            )
        )

    trace = bool(os.environ.get("MAMBA_TRACE"))
    if trace:
        _install_ntff_hook()
    nc = _get_nc()
    res = run_bass_kernel_spmd(nc, in_maps, core_ids=list(range(8)), trace=trace)
    LAST_EXEC_NS = res.exec_time_ns
    LAST_RESULTS = res

    h = x.astype(np.float32).copy()
    for i in range(8):
        y = res.results[i]["out"].T  # [L, D]
        if i >= B:
            y = y[::-1]
        h[i % B] += y
    mu = h.mean(axis=-1, keepdims=True, dtype=np.float64)
    var = np.mean((h - mu) ** 2, axis=-1, keepdims=True, dtype=np.float64)
    outp = (h - mu) / np.sqrt(var + 1e-5) * inputs["ln_w"] + inputs["ln_b"]
    return outp.astype(np.float32)


# revision 4
# speedup vs baseline: 1.0083x; 1.0003x over previous
"""Bidirectional Mamba block (B=4, L=1024, D=1024, DI=2048, DS=16) on 8
Trainium2 NeuronCores.

Sharding: one (batch, direction) pair per core - 8 fully data-parallel
shards, no collectives. Host flips the backward direction, sums the
residual and applies the final LayerNorm while gathering.

Per-core schedule (DVE-saturated; scans at the hardware 2cyc/elem floor):
- stage 1: x loaded as one contiguous all-port DMA; in_proj xh-half in
  fp8e4 DoubleRow (2x PE) + causal conv on DVE (fills the otherwise-
  idle pre-scan window) + silu + x_proj
- stage 3: z-half (fp8 DR) + silu gate batched in pairs of d-tiles (2
  ACT-table loads per pair), dt head, softplus via exp/ln (table 6),
  16 decay exps per d-tile on ACT, b/m elementwise muls batched 4
  states per wide 2x DVE op (broadcast-stride-0 du), selective scan on
  DVE, state-sum via identity-matmul PSUM accumulation on PE, gate
- out_proj in fp8e4 DoubleRow: first k-half woven under stage 3
  (partials spilled to DRAM); second half + merge software-pipelined
  (reload DMAs prefetched 3 ahead, all 8 PSUM banks)
- gpsimd stays idle during stage 3: it shares an SBUF port pair with
  the DVE and measurably slows the scans when active
"""

import os
import sys
import types

sys.path.insert(0, "/opt/trn_rl_repo")

import numpy as np
import ml_dtypes

BF16 = ml_dtypes.bfloat16
FP8 = ml_dtypes.float8_e4m3

import concourse.bass as bass
import concourse.mybir as mybir
from concourse.tile import TileContext
from concourse.bass_utils import run_bass_kernel_spmd
from concourse.masks import make_identity

P = 128
B, L, D = 4, 1024, 1024
DI, DS, DC, DR = 2048, 16, 4, 64
ND = DI // P          # 16 d-tiles
NK_D = D // P         # 8 k-tiles over D
NN = D // P           # 8 n-tiles of out_proj output
CH = 512              # psum chunk (free dim)
NCH = L // CH
NCOLS = 7 + DS        # conv_w(4), conv_b, dt_b, D, A(16)
G = 4                 # states per mul-group
NG = DS // G          # 4 groups

# engine split: which s-groups' muls run on gpsimd (rest on DVE, batched)
B_GP_GROUPS = ()          # gpsimd mul offload disabled (hurts DVE)
M_GP_GROUPS = ()

IN_FP8 = True         # in_proj (both halves) in fp8e4 DoubleRow
W_SCALE = 64.0        # fp8 weight scale
Y_SCALE = 8.0         # pre-gate y scale (fp8 opre dynamic range)

F32 = mybir.dt.float32
BF = mybir.dt.bfloat16
F8 = mybir.dt.float8e4
AF = mybir.ActivationFunctionType
OP = mybir.AluOpType

LAST_EXEC_NS = None
LAST_RESULTS = None


def _install_ntff_hook():
    import antenv

    if "antenv.axon_hooks" in sys.modules:
        return
    mod = types.ModuleType("antenv.axon_hooks")
    mod._hook = None
    mod.set_axon_ntff_profile_hook = lambda h: setattr(mod, "_hook", h)
    mod.get_axon_ntff_profile_hook = lambda: mod._hook
    sys.modules["antenv.axon_hooks"] = mod
    antenv.axon_hooks = mod
    try:
        from trn_agent_boot.trn_boot import _ntff_profile_via_ctypes

        mod.set_axon_ntff_profile_hook(
            _ntff_profile_via_ctypes("/opt/axon/libaxon_pjrt.so")
        )
    except Exception:
        pass


def split_excess_waits(nc, max_waits=1):
    """Walrus encodes at most `max_waits` sync-wait commands per instruction."""
    n_extra = 0
    for f in nc.m.functions:
        for bb in f.blocks:
            insts = bb.instructions
            i = 0
            while i < len(insts):
                inst = insts[i]
                si = inst.sync_info
                if si is not None and len(si.on_wait) > max_waits:
                    waits = list(si.on_wait)
                    for j, w in enumerate(waits[max_waits:]):
                        nop = mybir.InstNoOp(
                            name=f"{inst.name}-xw{j}",
                            engine=inst.engine,
                            bass_nofuse=True,
                            sync_info=mybir.SyncInfo(on_wait=[w], on_update=[]),
                        )
                        insts.insert(i, nop)
                        i += 1
                        n_extra += 1
                    inst.sync_info = mybir.SyncInfo(
                        on_wait=waits[:max_waits], on_update=list(si.on_update)
                    )
                i += 1
    return n_extra


def _build_program():
    nc = bass.Bass("TRN2")

    xdt = F8 if IN_FP8 else BF
    xt = nc.dram_tensor("xt", [P, NK_D * L], xdt, kind="ExternalInput")
    w_in = nc.dram_tensor("w_in", [2 * ND, P, NK_D // 2, 2, P], xdt, kind="ExternalInput")
    w_x = nc.dram_tensor("w_x", [P, ND * (DR + 2 * DS)], BF, kind="ExternalInput")
    w_dt = nc.dram_tensor("w_dt", [DR, ND * P], BF, kind="ExternalInput")
    w_out = nc.dram_tensor("w_out", [NN, P, ND // 2, 2, P], F8, kind="ExternalInput")
    chan = nc.dram_tensor("chan", [P, ND * NCOLS], F32, kind="ExternalInput")
    out = nc.dram_tensor("out", [D, L], F32, kind="ExternalOutput")

    xcb_scr = nc.dram_tensor("xcb_scr", [P, ND * L], BF)
    oh1_scr = nc.dram_tensor("oh1_scr", [P, NN, L], BF)
    bc_scr = nc.dram_tensor("bc_scr", [2 * DS, L], BF)

    with TileContext(nc) as tc:
        with tc.tile_pool(name="res", bufs=1) as res:
            Bc = res.tile([P, DS * L], BF, tag="Bc")
            Cc = res.tile([P, DS * L], BF, tag="Cc")
            ident = res.tile([P, P], BF, tag="ident")
            dbc_bf = res.tile([DR + 2 * DS, L], BF, tag="dbcbf")
            chan_all = res.tile([P, ND * NCOLS], F32, tag="chan")
            wx_all = res.tile([P, ND * (DR + 2 * DS)], BF, tag="wx")
            wdt_all = res.tile([DR, ND * P], BF, tag="wdt")
            kx = res.tile([P, NK_D, L], xdt, tag="kx")
            opre = res.tile([P, ND, L], F8, tag="opre")

            make_identity(nc, ident[:])
            nc.sync.dma_start(chan_all[:], chan[:])
            nc.sync.dma_start(wx_all[:], w_x[:])
            nc.sync.dma_start(wdt_all[:], w_dt[:])
            kxflat = kx[:].rearrange("p k l -> p (k l)")
            nc.sync.dma_start(kxflat, xt[:])

            def cc(m, col):
                return chan_all[:, m * NCOLS + col : m * NCOLS + col + 1]

            with tc.tile_pool(name="wi", bufs=3) as wip:
                # ---- stage 1: in_proj xh-half + conv(DVE) + silu + x_proj --
                with tc.tile_pool(name="s1", bufs=4) as s1p, \
                     tc.tile_pool(name="s1b", bufs=2) as s1q, \
                     tc.tile_pool(name="psh", bufs=3, space="PSUM") as pshp, \
                     tc.tile_pool(name="ps2", bufs=2, space="PSUM") as ps2p:
                    psx = [
                        ps2p.tile([DR + 2 * DS, CH], F32, tag="psx", name=f"psx{c}")
                        for c in range(NCH)
                    ]
                    for m in range(ND):
                        xh = s1q.tile([P, 3 + L], BF, tag="xh")
                        nc.gpsimd.memset(xh[:, 0:3], 0.0)
                        wi = wip.tile([P, NK_D // 2, 2, P], xdt, tag="wi", name=f"wia{m}")
                        nc.sync.dma_start(wi[:], w_in[m])
                        for c in range(NCH):
                            ps = pshp.tile([P, CH], F32, tag="psh")
                            if IN_FP8:
                                for kp in range(NK_D // 2):
                                    nc.tensor.matmul(
                                        ps[:],
                                        lhsT=wi[:, kp],
                                        rhs=kx[:, 2 * kp : 2 * kp + 2, c * CH : (c + 1) * CH],
                                        start=(kp == 0),
                                        stop=(kp == NK_D // 2 - 1),
                                        perf_mode=mybir.MatmulPerfMode.DoubleRow,
                                    )
                            else:
                                for k in range(NK_D):
                                    nc.tensor.matmul(
                                        ps[:],
                                        lhsT=wi[:, k // 2, k % 2],
                                        rhs=kx[:, k, c * CH : (c + 1) * CH],
                                        start=(k == 0),
                                        stop=(k == NK_D - 1),
                                    )
                            nc.scalar.activation(
                                xh[:, 3 + c * CH : 3 + (c + 1) * CH], ps[:], AF.Copy,
                                scale=(1.0 / W_SCALE) if IN_FP8 else 1.0,
                            )
                        # causal conv on DVE: acc = xh0*w0 + conv_b, 3 fused FMAs
                        acc0 = s1p.tile([P, L], F32, tag="accmid", name=f"ac0_{m}")
                        nc.vector.tensor_scalar(
                            out=acc0[:], in0=xh[:, 0:L],
                            scalar1=cc(m, 0), scalar2=cc(m, 4),
                            op0=OP.mult, op1=OP.add,
                        )
                        acc1 = s1p.tile([P, L], F32, tag="accmid", name=f"ac1_{m}")
                        nc.vector.scalar_tensor_tensor(
                            out=acc1[:], in0=xh[:, 1 : 1 + L], scalar=cc(m, 1),
                            in1=acc0[:], op0=OP.mult, op1=OP.add,
                        )
                        acc2 = s1p.tile([P, L], F32, tag="accmid", name=f"ac2_{m}")
                        nc.vector.scalar_tensor_tensor(
                            out=acc2[:], in0=xh[:, 2 : 2 + L], scalar=cc(m, 2),
                            in1=acc1[:], op0=OP.mult, op1=OP.add,
                        )
                        acc3 = s1p.tile([P, L], F32, tag="acc3", name=f"ac3_{m}")
                        nc.vector.scalar_tensor_tensor(
                            out=acc3[:], in0=xh[:, 3 : 3 + L], scalar=cc(m, 3),
                            in1=acc2[:], op0=OP.mult, op1=OP.add,
                        )
                        xcb = s1q.tile([P, L], BF, tag="xcb")
                        for c in range(NCH):
                            nc.scalar.activation(
                                xcb[:, c * CH : (c + 1) * CH],
                                acc3[:, c * CH : (c + 1) * CH], AF.Silu,
                            )
                            nc.tensor.matmul(
                                psx[c][:],
                                lhsT=wx_all[
                                    :, m * (DR + 2 * DS) : (m + 1) * (DR + 2 * DS)
                                ],
                                rhs=xcb[:, c * CH : (c + 1) * CH],
                                start=(m == 0),
                                stop=(m == ND - 1),
                            )
                        nc.sync.dma_start(xcb_scr[:, m * L : (m + 1) * L], xcb[:])
                    for c in range(NCH):
                        nc.scalar.activation(
                            dbc_bf[:, c * CH : (c + 1) * CH], psx[c][:], AF.Copy
                        )

                # ---- stage 2: broadcast B and C rows via replicated DMA ----
                nc.sync.dma_start(bc_scr[:], dbc_bf[DR : DR + 2 * DS, :])
                for s in range(DS):
                    nc.sync.dma_start(
                        Bc[:, s * L : (s + 1) * L],
                        bc_scr[s : s + 1, :].broadcast_to([P, L]),
                    )
                    nc.sync.dma_start(
                        Cc[:, s * L : (s + 1) * L],
                        bc_scr[DS + s : DS + s + 1, :].broadcast_to([P, L]),
                    )

                # ---- stage 3: z-half + dt head + scan + gate ---------------
                g_all = [None] * 4
                with tc.tile_pool(name="s3", bufs=2) as s3p, \
                     tc.tile_pool(name="s3g", bufs=4) as s3g, \
                     tc.tile_pool(name="s3a", bufs=2) as s3a, \
                     tc.tile_pool(name="s3b", bufs=2) as s3b, \
                     tc.tile_pool(name="s3h", bufs=1) as s3h, \
                     tc.tile_pool(name="s3m", bufs=1) as s3m, \
                     tc.tile_pool(name="s4w", bufs=2) as s4wp, \
                     tc.tile_pool(name="s4ho", bufs=2) as s4hop, \
                     tc.tile_pool(name="psd", bufs=1, space="PSUM") as psdp, \
                     tc.tile_pool(name="psz", bufs=2, space="PSUM") as pszp, \
                     tc.tile_pool(name="psh1", bufs=1, space="PSUM") as psh1p, \
                     tc.tile_pool(name="psy", bufs=2, space="PSUM") as psyp:
                    for m in range(ND):
                        # z-half matmuls + silu batched in quads of m so the
                        # silu<->exp/ln ACT-table switch happens 2x per 4 m
                        if m % 2 == 0:
                            for m2 in range(m, m + 2):
                                g_all[m2 % 4] = s3g.tile(
                                    [P, L], BF, tag="gm", name=f"g{m2}"
                                )
                                wiz = wip.tile(
                                    [P, NK_D // 2, 2, P], xdt, tag="wi", name=f"wiz{m2}"
                                )
                                nc.sync.dma_start(wiz[:], w_in[ND + m2])
                                for c in range(NCH):
                                    pz = pszp.tile([P, CH], F32, tag="psz")
                                    if IN_FP8:
                                        for kp in range(NK_D // 2):
                                            nc.tensor.matmul(
                                                pz[:],
                                                lhsT=wiz[:, kp],
                                                rhs=kx[:, 2 * kp : 2 * kp + 2, c * CH : (c + 1) * CH],
                                                start=(kp == 0),
                                                stop=(kp == NK_D // 2 - 1),
                                                perf_mode=mybir.MatmulPerfMode.DoubleRow,
                                            )
                                    else:
                                        for k in range(NK_D):
                                            nc.tensor.matmul(
                                                pz[:],
                                                lhsT=wiz[:, k // 2, k % 2],
                                                rhs=kx[:, k, c * CH : (c + 1) * CH],
                                                start=(k == 0),
                                                stop=(k == NK_D - 1),
                                            )
                                    nc.scalar.activation(
                                        g_all[m2 % 4][:, c * CH : (c + 1) * CH],
                                        pz[:], AF.Silu,
                                        scale=(1.0 / W_SCALE) if IN_FP8 else 1.0,
                                    )
                        g_m = g_all[m % 4]

                        xcb_m = s3p.tile([P, L], BF, tag="xcbm")
                        nc.sync.dma_start(xcb_m[:], xcb_scr[:, m * L : (m + 1) * L])

                        e_t = s3p.tile([P, L], F32, tag="e")
                        for c in range(NCH):
                            psd = psdp.tile([P, CH], F32, tag="psd", name=f"psd{m}_{c}")
                            nc.tensor.matmul(
                                psd[:],
                                lhsT=wdt_all[:, m * P : (m + 1) * P],
                                rhs=dbc_bf[0:DR, c * CH : (c + 1) * CH],
                                start=True,
                                stop=True,
                            )
                            nc.scalar.activation(
                                e_t[:, c * CH : (c + 1) * CH], psd[:], AF.Exp,
                                bias=cc(m, 5),
                            )
                        delta = s3p.tile([P, L], F32, tag="delta")
                        nc.scalar.activation(delta[:], e_t[:], AF.Ln, bias=1.0)
                        delta_b = s3p.tile([P, L], BF, tag="deltab")
                        nc.scalar.activation(delta_b[:], delta[:], AF.Copy)
                        du = s3p.tile([P, L], BF, tag="du")
                        nc.vector.tensor_mul(du[:], delta_b[:], xcb_m[:])

                        mD = s3p.tile([P, L], BF, tag="mD")
                        nc.scalar.activation(mD[:], xcb_m[:], AF.Copy, scale=cc(m, 6))
                        psy = psyp.tile([P, L], F32, tag="psy")
                        for c in range(NCH):
                            nc.tensor.matmul(
                                psy[:, c * CH : (c + 1) * CH],
                                lhsT=ident[:],
                                rhs=mD[:, c * CH : (c + 1) * CH],
                                start=True,
                                stop=False,
                            )
                        for q in range(NG):
                            s0 = q * G
                            agrp = s3a.tile([P, G * L], BF, tag="agrp")
                            for j in range(G):
                                nc.scalar.activation(
                                    agrp[:, j * L : (j + 1) * L], delta[:], AF.Exp,
                                    scale=cc(m, 7 + s0 + j),
                                )
                            bgrp = s3b.tile([P, G * L], BF, tag="bgrp")
                            nc.vector.tensor_mul(
                                bgrp[:],
                                du[:, None, :].broadcast_to([P, G, L]),
                                Bc[:, s0 * L : (s0 + G) * L],
                            )
                            hgrp = s3h.tile([P, G * L], BF, tag="hgrp")
                            for j in range(G):
                                nc.vector.tensor_tensor_scan(
                                    hgrp[:, j * L : (j + 1) * L],
                                    agrp[:, j * L : (j + 1) * L],
                                    bgrp[:, j * L : (j + 1) * L],
                                    0.0, op0=OP.mult, op1=OP.add,
                                )
                            mgrp = s3m.tile([P, G * L], BF, tag="mgrp")
                            nc.vector.tensor_mul(
                                mgrp[:], hgrp[:], Cc[:, s0 * L : (s0 + G) * L]
                            )
                            for j in range(G):
                                for c in range(NCH):
                                    nc.tensor.matmul(
                                        psy[:, c * CH : (c + 1) * CH],
                                        lhsT=ident[:],
                                        rhs=mgrp[:, j * L + c * CH : j * L + (c + 1) * CH],
                                        start=False,
                                        stop=(q == NG - 1 and j == G - 1),
                                    )
                        yb16 = s3p.tile([P, L], BF, tag="yb16")
                        nc.scalar.activation(yb16[:], psy[:], AF.Copy, scale=Y_SCALE)
                        og16 = s3p.tile([P, L], BF, tag="og16")
                        nc.vector.tensor_mul(og16[:], yb16[:], g_m[:])
                        nc.scalar.activation(opre[:, m, :], og16[:], AF.Copy)
                        if m >= NN:
                            # first k-half of out_proj for n = m - NN
                            n = m - NN
                            wo = s4wp.tile([P, ND // 2, 2, P], F8, tag="wo",
                                           name=f"wo{n}")
                            nc.sync.dma_start(wo[:], w_out[n])
                            oh1 = s4hop.tile([P, L], BF, tag="oh1", name=f"oh1_{n}")
                            for c in range(NCH):
                                ph = psh1p.tile([P, CH], F32, tag="ph",
                                                name=f"ph{n}_{c}")
                                for kp in range(ND // 4):
                                    nc.tensor.matmul(
                                        ph[:],
                                        lhsT=wo[:, kp],
                                        rhs=opre[:, 2 * kp : 2 * kp + 2,
                                                 c * CH : (c + 1) * CH],
                                        start=(kp == 0),
                                        stop=(kp == ND // 4 - 1),
                                        perf_mode=mybir.MatmulPerfMode.DoubleRow,
                                    )
                                nc.scalar.activation(
                                    oh1[:, c * CH : (c + 1) * CH], ph[:], AF.Copy,
                                    scale=1.0 / (W_SCALE * Y_SCALE),
                                )
                            nc.sync.dma_start(oh1_scr[:, n, :], oh1[:])

            # ------- stage 4: out_proj second k-half + merge with oh1 -------
            # software-pipelined: wo/oh1r DMAs issued 3 iterations ahead so
            # the merge chains never wait on the sync queue
            with tc.tile_pool(name="s4r", bufs=4) as s4r, \
                 tc.tile_pool(name="s4w2", bufs=4) as s4w2, \
                 tc.tile_pool(name="s4o", bufs=4) as s4o, \
                 tc.tile_pool(name="pso", bufs=8, space="PSUM") as psop:
                wos2 = [None] * NN
                oh1rs = [None] * NN
                PF = 3

                def _fetch(n):
                    wos2[n] = s4w2.tile([P, ND // 2, 2, P], F8, tag="wo2",
                                        name=f"wo2_{n}")
                    nc.sync.dma_start(wos2[n][:], w_out[n])
                    oh1rs[n] = s4r.tile([P, L], BF, tag="oh1r", name=f"oh1r{n}")
                    nc.sync.dma_start(oh1rs[n][:], oh1_scr[:, n, :])

                for n in range(min(PF, NN)):
                    _fetch(n)
                for n in range(NN):
                    if n + PF < NN:
                        _fetch(n + PF)
                    wo = wos2[n]
                    oh1r = oh1rs[n]
                    for c in range(NCH):
                        pso = psop.tile([P, CH], F32, tag="pso")
                        for kp in range(ND // 4, ND // 2):
                            nc.tensor.matmul(
                                pso[:],
                                lhsT=wo[:, kp],
                                rhs=opre[:, 2 * kp : 2 * kp + 2, c * CH : (c + 1) * CH],
                                start=(kp == ND // 4),
                                stop=(kp == ND // 2 - 1),
                                perf_mode=mybir.MatmulPerfMode.DoubleRow,
                            )
                        ob = s4o.tile([P, CH], F32, tag="ob")
                        nc.vector.scalar_tensor_tensor(
                            out=ob[:], in0=pso[:],
                            scalar=1.0 / (W_SCALE * Y_SCALE),
                            in1=oh1r[:, c * CH : (c + 1) * CH],
                            op0=OP.mult, op1=OP.add,
                        )
                        nc.sync.dma_start(
                            out[n * P : (n + 1) * P, c * CH : (c + 1) * CH], ob[:]
                        )

    split_excess_waits(nc)
    return nc


_NC = None


def _get_nc():
    global _NC
    if _NC is None:
        _NC = _build_program()
    return _NC


def _prep_core(x_b, flip, in_proj, conv_w, conv_b, x_proj, dt_w, dt_b, A_log, Dsk, out_proj):
    xdt_np = FP8 if IN_FP8 else BF16
    xtr = x_b[::-1].T if flip else x_b.T  # [D, L] fp32
    xt = np.ascontiguousarray(
        xtr.astype(xdt_np).reshape(NK_D, P, L).transpose(1, 0, 2)
    ).reshape(P, NK_D * L)

    w_in_f = in_proj.T.astype(np.float32)
    if IN_FP8:
        w_in_f = w_in_f * W_SCALE
    w_in_t = w_in_f.astype(xdt_np)  # [D, 2DI]
    # [k, P, 2ND, P] -> [2ND, P, kp, j, P] with k = 2*kp + j
    w_in = np.ascontiguousarray(
        w_in_t.reshape(NK_D // 2, 2, P, 2 * ND, P).transpose(3, 2, 0, 1, 4)
    )

    w_x_t = x_proj.T.astype(BF16)  # [DI, 96]
    w_x = np.ascontiguousarray(
        w_x_t.reshape(ND, P, DR + 2 * DS).transpose(1, 0, 2)
    ).reshape(P, ND * (DR + 2 * DS))

    w_dt = np.ascontiguousarray(dt_w.T.astype(BF16)).reshape(DR, ND * P)

    w_out_t = (out_proj.T.astype(np.float32) * W_SCALE).astype(FP8)  # [DI, D]
    w_out = np.ascontiguousarray(
        w_out_t.reshape(ND // 2, 2, P, NN, P).transpose(3, 2, 0, 1, 4)
    )

    A = -np.exp(A_log.astype(np.float64)).astype(np.float32)  # [DI, DS]
    chan_flat = np.concatenate(
        [
            conv_w.astype(np.float32),
            conv_b[:, None].astype(np.float32),
            dt_b[:, None].astype(np.float32),
            Dsk[:, None].astype(np.float32),
            A,
        ],
        axis=1,
    )
    chan = np.ascontiguousarray(
        chan_flat.reshape(ND, P, NCOLS).transpose(1, 0, 2)
    ).reshape(P, ND * NCOLS)

    return {
        "xt": xt,
        "w_in": w_in,
        "w_x": w_x,
        "w_dt": w_dt,
        "w_out": w_out,
        "chan": chan,
    }


def kernel(**inputs):
    global LAST_EXEC_NS, LAST_RESULTS
    inputs = {k: np.asarray(v) for k, v in inputs.items()}
    x = inputs["x"]

    in_maps = []
    for i in range(8):
        b = i % B
        p = "f" if i < B else "b"
        in_maps.append(
            _prep_core(
                x[b],
                flip=(p == "b"),
                in_proj=inputs[f"in_proj_{p}"],
                conv_w=inputs[f"conv_w_{p}"],
                conv_b=inputs[f"conv_b_{p}"],
                x_proj=inputs[f"x_proj_{p}"],
                dt_w=inputs[f"dt_w_{p}"],
                dt_b=inputs[f"dt_b_{p}"],
                A_log=inputs[f"A_log_{p}"],
                Dsk=inputs[f"D_{p}"],
                out_proj=inputs[f"out_proj_{p}"],
            )
        )

    trace = bool(os.environ.get("MAMBA_TRACE"))
    if trace:
        _install_ntff_hook()
    nc = _get_nc()
    res = run_bass_kernel_spmd(nc, in_maps, core_ids=list(range(8)), trace=trace)
    LAST_EXEC_NS = res.exec_time_ns
    LAST_RESULTS = res

    h = x.astype(np.float32).copy()
    for i in range(8):
        y = res.results[i]["out"].T  # [L, D]
        if i >= B:
            y = y[::-1]
        h[i % B] += y
    mu = h.mean(axis=-1, keepdims=True, dtype=np.float64)
    var = np.mean((h - mu) ** 2, axis=-1, keepdims=True, dtype=np.float64)
    outp = (h - mu) / np.sqrt(var + 1e-5) * inputs["ln_w"] + inputs["ln_b"]
    return outp.astype(np.float32)
